# revision 21
# baseline (speedup 1.0000x reference)
"""Trainium2 Bass kernel for BaseAttention (Bahdanau-style additive attention).

Reference computation (per batch row b):
    att_h  = h @ W.T + b_h                         # [B, A]
    dot    = tanh(iaf + att_h[:, None, :])         # [B, L, A]
    scores = dot @ alpha + alpha_b                 # [B, L]
    w      = softmax(scores, axis=1)               # [B, L]
    out    = sum_l w[b, l] * af[b, l, :]           # [B, D]

Sharding: data-parallel over batch, B=128 -> 16 per core across 8 cores.

The kernel is HBM-bound (aggregate DMA tops out ~330 GB/s/core), so every
large tensor is downcast to bf16 on the host (rel-err budget is 2e-2; bf16
costs ~3e-3) and pre-packed into partition-major layouts so each DMA line
is one long contiguous run. DMA count is minimized (each dma_start costs
~650 ns of serial descriptor generation on its issuing queue): all small
constants ride in one blob, and the af/iaf streams are issued from the
otherwise-idle GpSimd queue so they don't serialize behind the weights.

Math notes:
  - alpha_b drops out entirely (softmax is shift invariant).
  - b_h is folded into the h @ W.T matmul as an extra K=1 chunk whose lhs
    is a row of ones.
  - att_h broadcast to tile rows via an indicator matmul (ind_t.T @ att_hb)
    into PSUM; iaf is added in the same PSUM accumulation group by streaming
    it through the PE behind an identity lhsT (keeps the add off the DVE).
  - tanh straight out of PSUM; alpha-mul + reduce on DVE gives scores;
    softmax denominator deferred: e = exp(scores) unnormalized, the final
    result is (sum_l e*af) * 1/(sum_l e).
  - weighted sum over l is a matmul per (tile, d-chunk) using masked lhsT
    columns: e_cols[:, b] = e * indicator(row belongs to b); the denominator
    reuses e_cols against a constant [1, 0] column pair (free dim 2 keeps
    the 16-bit matmul free-dim rule happy).
  - the loop is software-pipelined with a 2-tile skew: iteration t runs the
    DVE/ACT score chain for tile t+1, the PSUM broadcast+inject for tile
    t+2, and the PE accumulation for tile t, so each engine's in-order
    queue always has ready work (the PE p-state ramp needs ~3 us of
    continuous busy to reach full clock).
"""

import os
from contextlib import ExitStack

import numpy as np
import ml_dtypes

import concourse.bass as bass
import concourse.mybir as mybir
import concourse.tile as tile
from concourse import bacc
from concourse.bass_utils import run_bass_kernel_spmd

F32 = mybir.dt.float32
BF16 = mybir.dt.bfloat16
FP8 = mybir.dt.float8e4
AF_T = mybir.ActivationFunctionType
NPBF16 = ml_dtypes.bfloat16
NPFP8 = ml_dtypes.float8_e4m3fn

B, L, D, A = 128, 196, 2048, 512
NCORES = 8
BPC = B // NCORES          # 16 batch rows per core
R = BPC * L                # 3136 (b, l) rows per core
P = 128                    # partitions
NT = (R + P - 1) // P      # 25 row tiles (24 full + one 64-row tail)
NFULL_T = R // P           # 24
TAILR = R - NFULL_T * P    # 64
KCH = D // P               # 16 k-chunks for the h @ W.T matmul
DCH = 4                    # d chunks of 512 for the weighted sum
DC = D // DCH              # 512
AFG = 5                    # row tiles per af/iaf DMA group (25 = 5*5)
NGRP = NT // AFG           # 5

# blob column offsets (bf16 elements)
HT_O = 0                   # [P, KCH*BPC]    packed h.T
ABC_O = HT_O + KCH * BPC   # [P, A]          alpha broadcast to all partitions
ID_O = ABC_O + A           # [P, P]          identity
IND_O = ID_O + P           # [P, NT*BPC]     packed row->batch indicator
BH_O = IND_O + NT * BPC    # [1, A]          h2att bias (partition 0)
ONES_O = BH_O + A          # [1, BPC]        ones (partition 0)
BLOB_C = ONES_O + BPC


def _ptile(t):
    return P if t < NT - 1 else TAILR


def _build_program():
    nc = bacc.Bacc(None, target_bir_lowering=False)

    w_t = nc.declare_dram_parameter("w_t", [P, KCH * A], BF16, isOutput=False)
    blob = nc.declare_dram_parameter("blob", [P, BLOB_C], BF16, isOutput=False)
    ind_t = nc.declare_dram_parameter("ind_t", [BPC, R], BF16, isOutput=False)
    iaf = nc.declare_dram_parameter("iaf", [P, NT * A], BF16, isOutput=False)
    af = nc.declare_dram_parameter("af", [P, NT * D], BF16, isOutput=False)
    out = nc.declare_dram_parameter("out", [BPC, D], F32, isOutput=True)

    with ExitStack() as ctx:
        tc = ctx.enter_context(tile.TileContext(nc))
        consts = ctx.enter_context(tc.tile_pool(name="consts", bufs=1))
        wpool = ctx.enter_context(tc.tile_pool(name="wpool", bufs=1))
        iafp = ctx.enter_context(tc.tile_pool(name="iafp", bufs=1))
        afp = ctx.enter_context(tc.tile_pool(name="afp", bufs=NGRP))
        scr = ctx.enter_context(tc.tile_pool(name="scr", bufs=3))
        ps_bc = ctx.enter_context(
            tc.tile_pool(name="ps_bc", bufs=3, space=bass.MemorySpace.PSUM)
        )
        ps_acc = ctx.enter_context(
            tc.tile_pool(name="ps_acc", bufs=1, space=bass.MemorySpace.PSUM)
        )

        # --- weights first on the Sync queue (everything gates on them),
        # then the const blob; af/iaf stream from the GpSimd queue ---
        blob_sb = consts.tile([P, BLOB_C], BF16)
        nc.sync.dma_start(blob_sb[:], blob[:, :])

        iaf_all = iafp.tile([P, NT * A], BF16)

        def issue_iaf(g):
            for t in range(g * AFG, (g + 1) * AFG):
                nc.sync.dma_start(
                    iaf_all[:, t * A : (t + 1) * A], iaf[:, t * A : (t + 1) * A]
                )

        issue_iaf(0)  # lands before w so the tile-0..2 injects run in the w window

        WCH = 4  # w arrives in 4 chunks so the att_h matmuls pipeline with it
        WKC = KCH // WCH
        w_sb = wpool.tile([P, KCH * A], BF16)
        for wc in range(WCH):
            nc.sync.dma_start(
                w_sb[:, wc * WKC * A : (wc + 1) * WKC * A],
                w_t[:, wc * WKC * A : (wc + 1) * WKC * A],
            )
        indt_sb = consts.tile([BPC, R], BF16)
        nc.sync.dma_start(indt_sb[:], ind_t[:, :])

        af_tiles_sb = {}

        def issue_group(g):
            # per-tile af DMAs: finer completion granularity keeps the PE fed
            if g > 0:
                issue_iaf(g)
            af_g = afp.tile([P, AFG * D], BF16, tag="af")
            for j in range(AFG):
                t = g * AFG + j
                nc.sync.dma_start(
                    af_g[:, j * D : (j + 1) * D], af[:, t * D : (t + 1) * D]
                )
                af_tiles_sb[t] = (af_g, j)

        for g in range(NGRP):
            issue_group(g)

        ones2_sb = consts.tile([P, 2], BF16)
        nc.gpsimd.memset(ones2_sb[:, 0:1], 1.0)
        nc.gpsimd.memset(ones2_sb[:, 1:2], 0.0)

        scores_all = consts.tile([P, NT], F32)
        e_all = consts.tile([P, NT], F32)

        # --- accumulators for the weighted sum and softmax denominator ---
        acc_ps = ps_acc.tile([BPC, DCH, DC], F32)
        sums_ps = ps_acc.tile([BPC, 2], F32)

        bc_tiles = {}

        def inject(t):
            """iaf streamed into a fresh PSUM group behind an identity lhsT."""
            pt = _ptile(t)
            bc_ps = ps_bc.tile([P, A], F32, tag="bc")
            nc.tensor.matmul(
                bc_ps[:pt, :],
                blob_sb[:pt, ID_O : ID_O + pt],
                iaf_all[:pt, t * A : (t + 1) * A],
                start=True,
                stop=False,
            )
            bc_tiles[t] = bc_ps

        def bcmm(t):
            """att_hb broadcast accumulated on top of the injected iaf."""
            pt = _ptile(t)
            nc.tensor.matmul(
                bc_tiles[t][:pt, :],
                indt_sb[:, t * P : t * P + pt],
                atthb_sb[:],
                start=False,
                stop=True,
            )

        def bc_inject(t):
            inject(t)
            bcmm(t)

        ecols_tiles = {}

        def chain_a(t):
            """tanh -> alpha-mul -> reduce (scores for tile t)."""
            pt = _ptile(t)
            bc_ps = bc_tiles.pop(t)
            tanh = scr.tile([P, A], BF16, tag="tanh")
            nc.scalar.activation(tanh[:pt, :], bc_ps[:pt, :], AF_T.Tanh)
            ttr_out = scr.tile([P, A], BF16, tag="ttr")
            nc.vector.tensor_mul(
                ttr_out[:pt, :], tanh[:pt, :], blob_sb[:pt, ABC_O : ABC_O + A]
            )
            nc.vector.tensor_reduce(
                scores_all[:pt, t : t + 1],
                ttr_out[:pt, :],
                axis=mybir.AxisListType.X,
                op=mybir.AluOpType.add,
            )

        def chain_b(t):
            """exp -> masked e columns (weights for tile t)."""
            pt = _ptile(t)
            nc.scalar.activation(
                e_all[:pt, t : t + 1], scores_all[:pt, t : t + 1], AF_T.Exp
            )
            ecols = scr.tile([P, BPC], BF16, tag="ecols", bufs=3)
            nc.vector.tensor_scalar_mul(
                ecols[:pt, :],
                blob_sb[:pt, IND_O + t * BPC : IND_O + (t + 1) * BPC],
                e_all[:pt, t : t + 1],
            )
            ecols_tiles[t] = ecols

        # tile-0/1 iaf injects run while w streams in
        inject(0)
        inject(1)

        # --- att_hb = h @ W.T + b_h (bias folded in as a K=1 chunk) ---
        atthb_ps = ps_bc.tile([BPC, A], F32, tag="bc")
        for k in range(KCH):
            nc.tensor.matmul(
                atthb_ps[:],
                blob_sb[:, HT_O + k * BPC : HT_O + (k + 1) * BPC],
                w_sb[:, k * A : (k + 1) * A],
                start=(k == 0),
                stop=False,
            )
        nc.tensor.matmul(
            atthb_ps[:],
            blob_sb[0:1, ONES_O : ONES_O + BPC],
            blob_sb[0:1, BH_O : BH_O + A],
            start=False,
            stop=True,
        )
        atthb_sb = consts.tile([BPC, A], BF16)
        nc.scalar.copy(atthb_sb[:], atthb_ps[:])

        bcmm(0)
        bcmm(1)
        chain_a(0)
        chain_a(1)
        chain_b(0)
        bc_inject(2)

        for t in range(NT):
            pt = _ptile(t)
            if t + 2 < NT:
                chain_a(t + 2)
            if t + 1 < NT:
                chain_b(t + 1)
            if t + 3 < NT:
                bc_inject(t + 3)

            af_g, af_j = af_tiles_sb.pop(t)
            ecols = ecols_tiles.pop(t)
            for c in range(DCH):
                nc.tensor.matmul(
                    acc_ps[:, c, :],
                    ecols[:pt, :],
                    af_g[:pt, af_j * D + c * DC : af_j * D + (c + 1) * DC],
                    start=(t == 0),
                    stop=(t == NT - 1),
                )
            # denominator: sums[b] = sum_rows e_cols[:, b]
            nc.tensor.matmul(
                sums_ps[:],
                ecols[:pt, :],
                ones2_sb[:pt, :],
                start=(t == 0),
                stop=(t == NT - 1),
            )

        # --- normalize and store ---
        recip = consts.tile([BPC, 1], F32)
        nc.vector.reciprocal(recip[:], sums_ps[:, 0:1])
        out_sb = consts.tile([BPC, D], F32)
        for c in range(DCH):
            if c % 2 == 0:
                nc.scalar.mul(out_sb[:, c * DC : (c + 1) * DC], acc_ps[:, c, :], recip[:])
            else:
                nc.vector.tensor_scalar_mul(
                    out_sb[:, c * DC : (c + 1) * DC], acc_ps[:, c, :], recip[:]
                )
            nc.sync.dma_start(
                out[:, c * DC : (c + 1) * DC], out_sb[:, c * DC : (c + 1) * DC]
            )

    nc.compile()
    return nc


_PROGRAM = None


def _get_program():
    global _PROGRAM
    if _PROGRAM is None:
        _PROGRAM = _build_program()
    return _PROGRAM


def _pack_rows(x16, ncols):
    """[R, C] -> [P, NT*C] with dev[p, t*C:(t+1)*C] = x[t*P+p], zero pad."""
    dev = np.zeros((P, NT, ncols), x16.dtype)
    dev[:, :NFULL_T, :] = x16[: NFULL_T * P].reshape(NFULL_T, P, ncols).transpose(1, 0, 2)
    dev[:TAILR, NFULL_T, :] = x16[NFULL_T * P :]
    return np.ascontiguousarray(dev.reshape(P, NT * ncols))


def _host_prep(h, att_feats, internal_att_feats, h2att_w, h2att_b, alpha_w, alpha_b):
    h16 = np.asarray(h).astype(NPBF16)
    af16 = np.asarray(att_feats).astype(NPBF16).reshape(B, L * D)
    iaf16 = np.asarray(internal_att_feats).astype(NPBF16).reshape(B, L * A)
    w16 = np.asarray(h2att_w).astype(NPBF16)                    # [A, D]

    # w_dev[p, k*A+a] = W[a, k*P+p]
    w_dev = np.ascontiguousarray(
        w16.T.reshape(KCH, P, A).transpose(1, 0, 2).reshape(P, KCH * A)
    )

    ind_rows = np.zeros((NT * P, BPC), np.float32)
    rows = np.arange(R)
    ind_rows[rows, rows // L] = 1.0
    ind_t = np.ascontiguousarray(ind_rows[:R].T).astype(NPBF16)  # [BPC, R]
    ind_dev = ind_rows.reshape(NT, P, BPC).transpose(1, 0, 2).reshape(P, NT * BPC)

    blob_common = np.zeros((P, BLOB_C), NPBF16)
    blob_common[:, ABC_O : ABC_O + A] = np.asarray(alpha_w, np.float32).reshape(1, A)
    blob_common[:, ID_O : ID_O + P] = np.eye(P, dtype=NPBF16)
    blob_common[:, IND_O : IND_O + NT * BPC] = ind_dev
    blob_common[0, BH_O : BH_O + A] = np.asarray(h2att_b, np.float32).reshape(A)
    blob_common[0, ONES_O : ONES_O + BPC] = 1.0

    in_maps = []
    for i in range(NCORES):
        sl = slice(i * BPC, (i + 1) * BPC)
        blob_i = blob_common.copy()
        # h_t_dev[p, k*BPC+b] = h[b, k*P+p]
        blob_i[:, HT_O : HT_O + KCH * BPC] = (
            h16[sl].T.reshape(KCH, P, BPC).transpose(1, 0, 2).reshape(P, KCH * BPC)
        )
        in_maps.append(
            {
                "w_t": w_dev,
                "blob": blob_i,
                "ind_t": ind_t,
                "iaf": _pack_rows(iaf16[sl].reshape(R, A), A),
                "af": _pack_rows(af16[sl].reshape(R, D), D),
            }
        )
    return in_maps


def run(trace=False, **inputs):
    """Run the SPMD kernel; returns (full_output [B, D], BassKernelResults)."""
    nc = _get_program()
    in_maps = _host_prep(**inputs)
    res = run_bass_kernel_spmd(nc, in_maps, list(range(NCORES)), trace=trace)
    out = np.concatenate([res.results[i]["out"] for i in range(NCORES)], axis=0)
    return out, res


def kernel(**inputs):
    out, _ = run(trace=False, **inputs)
    return out


# revision 23
# speedup vs baseline: 1.0884x; 1.0884x over previous
"""Trainium2 Bass kernel for BaseAttention (Bahdanau-style additive attention).

Reference computation (per batch row b):
    att_h  = h @ W.T + b_h                         # [B, A]
    dot    = tanh(iaf + att_h[:, None, :])         # [B, L, A]
    scores = dot @ alpha + alpha_b                 # [B, L]
    w      = softmax(scores, axis=1)               # [B, L]
    out    = sum_l w[b, l] * af[b, l, :]           # [B, D]

Sharding: data-parallel over batch, B=128 -> 16 per core across 8 cores.

The kernel is HBM-bound (aggregate DMA tops out ~330 GB/s/core), so every
large tensor is downcast to bf16 on the host (rel-err budget is 2e-2; bf16
costs ~3e-3) and pre-packed into partition-major layouts so each DMA line
is one long contiguous run. All DMAs ride one queue in priority order
(blob, iaf chunk 0, w in 4 pipelined chunks, then the whole af/iaf stream
up front) — the DGE drains descriptors roughly in order, so ordering is
the prefetch policy and the stream never competes with the weights.

Math notes:
  - alpha_b drops out entirely (softmax is shift invariant).
  - b_h is folded into the h @ W.T matmul as an extra K=1 chunk whose lhs
    is a row of ones.
  - att_h broadcast to tile rows via an indicator matmul (ind_t.T @ att_hb)
    into PSUM; iaf is added in the same PSUM accumulation group by streaming
    it through the PE behind an identity lhsT (keeps the add off the DVE).
  - tanh straight out of PSUM; alpha-mul + reduce on DVE gives scores;
    softmax denominator deferred: e = exp(scores) unnormalized, the final
    result is (sum_l e*af) * 1/(sum_l e).
  - weighted sum over l is a matmul per (tile, d-chunk) using masked lhsT
    columns: e_cols[:, b] = e * indicator(row belongs to b); the denominator
    reuses e_cols against a constant [1, 0] column pair (free dim 2 keeps
    the 16-bit matmul free-dim rule happy).
  - the loop is software-pipelined with a multi-tile skew: iteration t
    runs tanh/mul/reduce for tile t+2, exp/e-columns for tile t+1, the
    PSUM inject+broadcast for tile t+3, and the PE accumulation for tile
    t, so every cross-engine edge has at least one iteration of slack and
    no in-order engine queue stalls (the PE p-state ramp needs ~3 us of
    continuous busy to reach full clock).
"""

import os
from contextlib import ExitStack

import numpy as np
import ml_dtypes

import concourse.bass as bass
import concourse.mybir as mybir
import concourse.tile as tile
from concourse import bacc
from concourse.bass_utils import run_bass_kernel_spmd

F32 = mybir.dt.float32
BF16 = mybir.dt.bfloat16
FP8 = mybir.dt.float8e4
AF_T = mybir.ActivationFunctionType
NPBF16 = ml_dtypes.bfloat16
NPFP8 = ml_dtypes.float8_e4m3fn

B, L, D, A = 128, 196, 2048, 512
NCORES = 8
BPC = B // NCORES          # 16 batch rows per core
R = BPC * L                # 3136 (b, l) rows per core
P = 128                    # partitions
NT = (R + P - 1) // P      # 25 row tiles (24 full + one 64-row tail)
NFULL_T = R // P           # 24
TAILR = R - NFULL_T * P    # 64
KCH = D // P               # 16 k-chunks for the h @ W.T matmul
DCH = 4                    # d chunks of 512 for the weighted sum
DC = D // DCH              # 512
AFG = 5                    # row tiles per af/iaf DMA group (25 = 5*5)
NGRP = NT // AFG           # 5

# blob column offsets (bf16 elements)
HT_O = 0                   # [P, KCH*BPC]    packed h.T
ABC_O = HT_O + KCH * BPC   # [P, A]          alpha broadcast to all partitions
ID_O = ABC_O + A           # [P, P]          identity
IND_O = ID_O + P           # [P, NT*BPC]     packed row->batch indicator
BH_O = IND_O + NT * BPC    # [1, A]          h2att bias (partition 0)
ONES_O = BH_O + A          # [1, BPC]        ones (partition 0)
BLOB_C = ONES_O + BPC


def _ptile(t):
    return P if t < NT - 1 else TAILR


def _build_program():
    nc = bacc.Bacc(None, target_bir_lowering=False)

    w_t = nc.declare_dram_parameter("w_t", [P, KCH * A], BF16, isOutput=False)
    blob = nc.declare_dram_parameter("blob", [P, BLOB_C], BF16, isOutput=False)
    ind_t = nc.declare_dram_parameter("ind_t", [BPC, R], BF16, isOutput=False)
    iaf = nc.declare_dram_parameter("iaf", [P, NT * A], BF16, isOutput=False)
    af = nc.declare_dram_parameter("af", [P, NT * D], BF16, isOutput=False)
    out = nc.declare_dram_parameter("out", [BPC, D], F32, isOutput=True)

    with ExitStack() as ctx:
        tc = ctx.enter_context(tile.TileContext(nc))
        consts = ctx.enter_context(tc.tile_pool(name="consts", bufs=1))
        wpool = ctx.enter_context(tc.tile_pool(name="wpool", bufs=1))
        iafp = ctx.enter_context(tc.tile_pool(name="iafp", bufs=1))
        afp = ctx.enter_context(tc.tile_pool(name="afp", bufs=NGRP))
        scr = ctx.enter_context(tc.tile_pool(name="scr", bufs=2))
        ps_bc = ctx.enter_context(
            tc.tile_pool(name="ps_bc", bufs=3, space=bass.MemorySpace.PSUM)
        )
        ps_acc = ctx.enter_context(
            tc.tile_pool(name="ps_acc", bufs=1, space=bass.MemorySpace.PSUM)
        )

        # --- weights first on the Sync queue (everything gates on them),
        # then the const blob; af/iaf stream from the GpSimd queue ---
        blob_sb = consts.tile([P, BLOB_C], BF16)
        nc.sync.dma_start(blob_sb[:], blob[:, :])

        iaf_all = iafp.tile([P, NT * A], BF16)

        def issue_iaf(g):
            t0 = g * AFG
            nc.sync.dma_start(
                iaf_all[:, t0 * A : (t0 + AFG) * A], iaf[:, t0 * A : (t0 + AFG) * A]
            )

        issue_iaf(0)  # lands before w so the tile-0..2 injects run in the w window

        WCH = 4  # w arrives in 4 chunks so the att_h matmuls pipeline with it
        WKC = KCH // WCH
        w_sb = wpool.tile([P, KCH * A], BF16)
        for wc in range(WCH):
            nc.sync.dma_start(
                w_sb[:, wc * WKC * A : (wc + 1) * WKC * A],
                w_t[:, wc * WKC * A : (wc + 1) * WKC * A],
            )
        indt_sb = consts.tile([BPC, R], BF16)
        nc.sync.dma_start(indt_sb[:], ind_t[:, :])

        af_tiles_sb = {}

        def issue_group(g):
            # per-tile af DMAs: finer completion granularity keeps the PE fed
            if g > 0:
                issue_iaf(g)
            af_g = afp.tile([P, AFG * D], BF16, tag="af")
            for j in range(AFG):
                t = g * AFG + j
                nc.sync.dma_start(
                    af_g[:, j * D : (j + 1) * D], af[:, t * D : (t + 1) * D]
                )
                af_tiles_sb[t] = (af_g, j)

        for g in range(NGRP):
            issue_group(g)

        ones2_sb = consts.tile([P, 2], BF16)
        nc.gpsimd.memset(ones2_sb[:, 0:1], 1.0)
        nc.gpsimd.memset(ones2_sb[:, 1:2], 0.0)

        scores_all = consts.tile([P, NT], F32)
        e_all = consts.tile([P, NT], F32)

        # --- accumulators for the weighted sum and softmax denominator ---
        acc_ps = ps_acc.tile([BPC, DCH, DC], F32)
        sums_ps = ps_acc.tile([BPC, 2], F32)

        bc_tiles = {}

        def inject(t):
            """iaf streamed into a fresh PSUM group behind an identity lhsT."""
            pt = _ptile(t)
            bc_ps = ps_bc.tile([P, A], F32, tag="bc")
            nc.tensor.matmul(
                bc_ps[:pt, :],
                blob_sb[:pt, ID_O : ID_O + pt],
                iaf_all[:pt, t * A : (t + 1) * A],
                start=True,
                stop=False,
            )
            bc_tiles[t] = bc_ps

        def bcmm(t):
            """att_hb broadcast accumulated on top of the injected iaf."""
            pt = _ptile(t)
            nc.tensor.matmul(
                bc_tiles[t][:pt, :],
                indt_sb[:, t * P : t * P + pt],
                atthb_sb[:],
                start=False,
                stop=True,
            )

        def bc_inject(t):
            inject(t)
            bcmm(t)

        ecols_tiles = {}

        def chain_a(t):
            """tanh -> alpha-mul -> reduce (scores for tile t)."""
            pt = _ptile(t)
            bc_ps = bc_tiles.pop(t)
            tanh = scr.tile([P, A], BF16, tag="tanh")
            nc.scalar.activation(tanh[:pt, :], bc_ps[:pt, :], AF_T.Tanh)
            ttr_out = scr.tile([P, A], BF16, tag="ttr")
            nc.vector.tensor_mul(
                ttr_out[:pt, :], tanh[:pt, :], blob_sb[:pt, ABC_O : ABC_O + A]
            )
            nc.vector.tensor_reduce(
                scores_all[:pt, t : t + 1],
                ttr_out[:pt, :],
                axis=mybir.AxisListType.X,
                op=mybir.AluOpType.add,
            )

        def chain_b(t):
            """exp -> masked e columns (weights for tile t)."""
            pt = _ptile(t)
            nc.scalar.activation(
                e_all[:pt, t : t + 1], scores_all[:pt, t : t + 1], AF_T.Exp
            )
            ecols = scr.tile([P, BPC], BF16, tag="ecols", bufs=3)
            nc.vector.tensor_scalar_mul(
                ecols[:pt, :],
                blob_sb[:pt, IND_O + t * BPC : IND_O + (t + 1) * BPC],
                e_all[:pt, t : t + 1],
            )
            ecols_tiles[t] = ecols

        # tile-0/1 iaf injects run while w streams in
        inject(0)
        inject(1)

        # --- att_hb = h @ W.T + b_h (bias folded in as a K=1 chunk) ---
        atthb_ps = ps_bc.tile([BPC, A], F32, tag="bc")
        for k in range(KCH):
            nc.tensor.matmul(
                atthb_ps[:],
                blob_sb[:, HT_O + k * BPC : HT_O + (k + 1) * BPC],
                w_sb[:, k * A : (k + 1) * A],
                start=(k == 0),
                stop=False,
            )
        nc.tensor.matmul(
            atthb_ps[:],
            blob_sb[0:1, ONES_O : ONES_O + BPC],
            blob_sb[0:1, BH_O : BH_O + A],
            start=False,
            stop=True,
        )
        atthb_sb = consts.tile([BPC, A], BF16)
        nc.scalar.copy(atthb_sb[:], atthb_ps[:])

        bcmm(0)
        bcmm(1)
        chain_a(0)
        chain_a(1)
        chain_b(0)
        bc_inject(2)

        for t in range(NT):
            pt = _ptile(t)
            if t + 2 < NT:
                chain_a(t + 2)
            if t + 1 < NT:
                chain_b(t + 1)
            if t + 3 < NT:
                bc_inject(t + 3)

            af_g, af_j = af_tiles_sb.pop(t)
            ecols = ecols_tiles.pop(t)
            for c in range(DCH):
                nc.tensor.matmul(
                    acc_ps[:, c, :],
                    ecols[:pt, :],
                    af_g[:pt, af_j * D + c * DC : af_j * D + (c + 1) * DC],
                    start=(t == 0),
                    stop=(t == NT - 1),
                )
            # denominator: sums[b] = sum_rows e_cols[:, b]
            nc.tensor.matmul(
                sums_ps[:],
                ecols[:pt, :],
                ones2_sb[:pt, :],
                start=(t == 0),
                stop=(t == NT - 1),
            )

        # --- normalize and store ---
        recip = consts.tile([BPC, 1], F32)
        nc.vector.reciprocal(recip[:], sums_ps[:, 0:1])
        out_sb = consts.tile([BPC, D], F32)
        for c in range(DCH):
            if c % 2 == 0:
                nc.scalar.mul(out_sb[:, c * DC : (c + 1) * DC], acc_ps[:, c, :], recip[:])
            else:
                nc.vector.tensor_scalar_mul(
                    out_sb[:, c * DC : (c + 1) * DC], acc_ps[:, c, :], recip[:]
                )
            nc.sync.dma_start(
                out[:, c * DC : (c + 1) * DC], out_sb[:, c * DC : (c + 1) * DC]
            )

    nc.compile()
    return nc


_PROGRAM = None


def _get_program():
    global _PROGRAM
    if _PROGRAM is None:
        _PROGRAM = _build_program()
    return _PROGRAM


def _pack_rows(x16, ncols):
    """[R, C] -> [P, NT*C] with dev[p, t*C:(t+1)*C] = x[t*P+p], zero pad."""
    dev = np.zeros((P, NT, ncols), x16.dtype)
    dev[:, :NFULL_T, :] = x16[: NFULL_T * P].reshape(NFULL_T, P, ncols).transpose(1, 0, 2)
    dev[:TAILR, NFULL_T, :] = x16[NFULL_T * P :]
    return np.ascontiguousarray(dev.reshape(P, NT * ncols))


def _host_prep(h, att_feats, internal_att_feats, h2att_w, h2att_b, alpha_w, alpha_b):
    h16 = np.asarray(h).astype(NPBF16)
    af16 = np.asarray(att_feats).astype(NPBF16).reshape(B, L * D)
    iaf16 = np.asarray(internal_att_feats).astype(NPBF16).reshape(B, L * A)
    w16 = np.asarray(h2att_w).astype(NPBF16)                    # [A, D]

    # w_dev[p, k*A+a] = W[a, k*P+p]
    w_dev = np.ascontiguousarray(
        w16.T.reshape(KCH, P, A).transpose(1, 0, 2).reshape(P, KCH * A)
    )

    ind_rows = np.zeros((NT * P, BPC), np.float32)
    rows = np.arange(R)
    ind_rows[rows, rows // L] = 1.0
    ind_t = np.ascontiguousarray(ind_rows[:R].T).astype(NPBF16)  # [BPC, R]
    ind_dev = ind_rows.reshape(NT, P, BPC).transpose(1, 0, 2).reshape(P, NT * BPC)

    blob_common = np.zeros((P, BLOB_C), NPBF16)
    blob_common[:, ABC_O : ABC_O + A] = np.asarray(alpha_w, np.float32).reshape(1, A)
    blob_common[:, ID_O : ID_O + P] = np.eye(P, dtype=NPBF16)
    blob_common[:, IND_O : IND_O + NT * BPC] = ind_dev
    blob_common[0, BH_O : BH_O + A] = np.asarray(h2att_b, np.float32).reshape(A)
    blob_common[0, ONES_O : ONES_O + BPC] = 1.0

    in_maps = []
    for i in range(NCORES):
        sl = slice(i * BPC, (i + 1) * BPC)
        blob_i = blob_common.copy()
        # h_t_dev[p, k*BPC+b] = h[b, k*P+p]
        blob_i[:, HT_O : HT_O + KCH * BPC] = (
            h16[sl].T.reshape(KCH, P, BPC).transpose(1, 0, 2).reshape(P, KCH * BPC)
        )
        in_maps.append(
            {
                "w_t": w_dev,
                "blob": blob_i,
                "ind_t": ind_t,
                "iaf": _pack_rows(iaf16[sl].reshape(R, A), A),
                "af": _pack_rows(af16[sl].reshape(R, D), D),
            }
        )
    return in_maps


def run(trace=False, **inputs):
    """Run the SPMD kernel; returns (full_output [B, D], BassKernelResults)."""
    nc = _get_program()
    in_maps = _host_prep(**inputs)
    res = run_bass_kernel_spmd(nc, in_maps, list(range(NCORES)), trace=trace)
    out = np.concatenate([res.results[i]["out"] for i in range(NCORES)], axis=0)
    return out, res


def kernel(**inputs):
    out, _ = run(trace=False, **inputs)
    return out


# revision 24
# speedup vs baseline: 1.0942x; 1.0053x over previous
"""Trainium2 Bass kernel for BaseAttention (Bahdanau-style additive attention).

Reference computation (per batch row b):
    att_h  = h @ W.T + b_h                         # [B, A]
    dot    = tanh(iaf + att_h[:, None, :])         # [B, L, A]
    scores = dot @ alpha + alpha_b                 # [B, L]
    w      = softmax(scores, axis=1)               # [B, L]
    out    = sum_l w[b, l] * af[b, l, :]           # [B, D]

Sharding: data-parallel over batch, B=128 -> 16 per core across 8 cores.

The kernel is HBM-bound (aggregate DMA tops out ~330 GB/s/core), so every
large tensor is downcast to bf16 on the host (rel-err budget is 2e-2; bf16
costs ~3e-3) and pre-packed into partition-major layouts so each DMA line
is one long contiguous run. All DMAs ride one queue in priority order
(blob, iaf chunk 0, w in 4 pipelined chunks, then the whole af/iaf stream
up front) — the DGE drains descriptors roughly in order, so ordering is
the prefetch policy and the stream never competes with the weights.

Math notes:
  - alpha_b drops out entirely (softmax is shift invariant).
  - b_h is folded into the h @ W.T matmul as an extra K=1 chunk whose lhs
    is a row of ones.
  - att_h broadcast to tile rows via an indicator matmul (ind_t.T @ att_hb)
    into PSUM; iaf is added in the same PSUM accumulation group by streaming
    it through the PE behind an identity lhsT (keeps the add off the DVE).
  - tanh straight out of PSUM; alpha-mul + reduce on DVE gives scores;
    softmax denominator deferred: e = exp(scores) unnormalized, the final
    result is (sum_l e*af) * 1/(sum_l e).
  - weighted sum over l is a matmul per (tile, d-chunk) using masked lhsT
    columns: e_cols[:, b] = e * indicator(row belongs to b); the denominator
    reuses e_cols against a constant [1, 0] column pair (free dim 2 keeps
    the 16-bit matmul free-dim rule happy).
  - the loop is software-pipelined with a multi-tile skew: iteration t
    runs tanh/mul/reduce for tile t+2, exp/e-columns for tile t+1, the
    PSUM inject+broadcast for tile t+3, and the PE accumulation for tile
    t, so every cross-engine edge has at least one iteration of slack and
    no in-order engine queue stalls (the PE p-state ramp needs ~3 us of
    continuous busy to reach full clock).
"""

import os
from contextlib import ExitStack

import numpy as np
import ml_dtypes

import concourse.bass as bass
import concourse.mybir as mybir
import concourse.tile as tile
from concourse import bacc
from concourse.bass_utils import run_bass_kernel_spmd

F32 = mybir.dt.float32
BF16 = mybir.dt.bfloat16
FP8 = mybir.dt.float8e4
AF_T = mybir.ActivationFunctionType
NPBF16 = ml_dtypes.bfloat16
NPFP8 = ml_dtypes.float8_e4m3fn

B, L, D, A = 128, 196, 2048, 512
NCORES = 8
BPC = B // NCORES          # 16 batch rows per core
R = BPC * L                # 3136 (b, l) rows per core
P = 128                    # partitions
NT = (R + P - 1) // P      # 25 row tiles (24 full + one 64-row tail)
NFULL_T = R // P           # 24
TAILR = R - NFULL_T * P    # 64
KCH = D // P               # 16 k-chunks for the h @ W.T matmul
DCH = 4                    # d chunks of 512 for the weighted sum
DC = D // DCH              # 512
AFG = 5                    # row tiles per af/iaf DMA group (25 = 5*5)
NGRP = NT // AFG           # 5

# blob column offsets (bf16 elements)
HT_O = 0                   # [P, KCH*BPC]    packed h.T
ABC_O = HT_O + KCH * BPC   # [P, A]          alpha broadcast to all partitions
ID_O = ABC_O + A           # [P, P]          identity
IND_O = ID_O + P           # [P, NT*BPC]     packed row->batch indicator
BH_O = IND_O + NT * BPC    # [1, A]          h2att bias (partition 0)
ONES_O = BH_O + A          # [1, BPC]        ones (partition 0)
BLOB_C = ONES_O + BPC


def _ptile(t):
    return P if t < NT - 1 else TAILR


def _build_program():
    nc = bacc.Bacc(None, target_bir_lowering=False)

    w_t = nc.declare_dram_parameter("w_t", [P, KCH * A], BF16, isOutput=False)
    blob = nc.declare_dram_parameter("blob", [P, BLOB_C], BF16, isOutput=False)
    ind_t = nc.declare_dram_parameter("ind_t", [BPC, R], BF16, isOutput=False)
    iaf = nc.declare_dram_parameter("iaf", [P, NT * A], BF16, isOutput=False)
    af = nc.declare_dram_parameter("af", [P, NT * D], BF16, isOutput=False)
    out = nc.declare_dram_parameter("out", [BPC, D], F32, isOutput=True)

    with ExitStack() as ctx:
        tc = ctx.enter_context(tile.TileContext(nc))
        consts = ctx.enter_context(tc.tile_pool(name="consts", bufs=1))
        wpool = ctx.enter_context(tc.tile_pool(name="wpool", bufs=1))
        iafp = ctx.enter_context(tc.tile_pool(name="iafp", bufs=1))
        afp = ctx.enter_context(tc.tile_pool(name="afp", bufs=NGRP))
        scr = ctx.enter_context(tc.tile_pool(name="scr", bufs=2))
        ps_bc = ctx.enter_context(
            tc.tile_pool(name="ps_bc", bufs=3, space=bass.MemorySpace.PSUM)
        )
        ps_acc = ctx.enter_context(
            tc.tile_pool(name="ps_acc", bufs=1, space=bass.MemorySpace.PSUM)
        )

        # --- weights first on the Sync queue (everything gates on them),
        # then the const blob; af/iaf stream from the GpSimd queue ---
        blob_sb = consts.tile([P, BLOB_C], BF16)
        nc.sync.dma_start(blob_sb[:], blob[:, :])

        iaf_all = iafp.tile([P, NT * A], BF16)

        def issue_iaf(g):
            t0 = g * AFG
            nc.sync.dma_start(
                iaf_all[:, t0 * A : (t0 + AFG) * A], iaf[:, t0 * A : (t0 + AFG) * A]
            )

        issue_iaf(0)  # lands before w so the tile-0..2 injects run in the w window

        WCH = 4  # w arrives in 4 chunks so the att_h matmuls pipeline with it
        WKC = KCH // WCH
        w_sb = wpool.tile([P, KCH * A], BF16)
        for wc in range(WCH):
            nc.sync.dma_start(
                w_sb[:, wc * WKC * A : (wc + 1) * WKC * A],
                w_t[:, wc * WKC * A : (wc + 1) * WKC * A],
            )
        indt_sb = consts.tile([BPC, R], BF16)
        nc.sync.dma_start(indt_sb[:], ind_t[:, :])

        af_tiles_sb = {}

        def issue_group(g):
            # per-tile af DMAs: finer completion granularity keeps the PE fed
            if g > 0:
                issue_iaf(g)
            af_g = afp.tile([P, AFG * D], BF16, tag="af")
            for j in range(AFG):
                t = g * AFG + j
                nc.sync.dma_start(
                    af_g[:, j * D : (j + 1) * D], af[:, t * D : (t + 1) * D]
                )
                af_tiles_sb[t] = (af_g, j)

        for g in range(NGRP):
            issue_group(g)

        ones2_sb = consts.tile([P, 2], BF16)
        nc.gpsimd.memset(ones2_sb[:, 0:1], 1.0)
        nc.gpsimd.memset(ones2_sb[:, 1:2], 0.0)

        scores_all = consts.tile([P, NT], F32)
        e_all = consts.tile([P, NT], F32)

        # --- accumulators for the weighted sum and softmax denominator ---
        acc_ps = ps_acc.tile([BPC, DCH, DC], F32)
        sums_ps = ps_acc.tile([BPC, 2], F32)

        bc_tiles = {}

        def inject(t):
            """iaf streamed into a fresh PSUM group behind an identity lhsT."""
            pt = _ptile(t)
            bc_ps = ps_bc.tile([P, A], F32, tag="bc")
            nc.tensor.matmul(
                bc_ps[:pt, :],
                blob_sb[:pt, ID_O : ID_O + pt],
                iaf_all[:pt, t * A : (t + 1) * A],
                start=True,
                stop=False,
            )
            bc_tiles[t] = bc_ps

        def bcmm(t):
            """att_hb broadcast accumulated on top of the injected iaf."""
            pt = _ptile(t)
            nc.tensor.matmul(
                bc_tiles[t][:pt, :],
                indt_sb[:, t * P : t * P + pt],
                atthb_sb[:],
                start=False,
                stop=True,
            )

        def bc_inject(t):
            inject(t)
            bcmm(t)

        ecols_tiles = {}

        def chain_a(t):
            """tanh -> alpha-mul -> reduce (scores for tile t)."""
            pt = _ptile(t)
            bc_ps = bc_tiles.pop(t)
            tanh = scr.tile([P, A], BF16, tag="tanh")
            nc.scalar.activation(tanh[:pt, :], bc_ps[:pt, :], AF_T.Tanh)
            ttr_out = scr.tile([P, A], BF16, tag="ttr")
            nc.vector.tensor_mul(
                ttr_out[:pt, :], tanh[:pt, :], blob_sb[:pt, ABC_O : ABC_O + A]
            )
            nc.vector.tensor_reduce(
                scores_all[:pt, t : t + 1],
                ttr_out[:pt, :],
                axis=mybir.AxisListType.X,
                op=mybir.AluOpType.add,
            )

        def chain_b(t):
            """exp -> masked e columns (weights for tile t)."""
            pt = _ptile(t)
            nc.scalar.activation(
                e_all[:pt, t : t + 1], scores_all[:pt, t : t + 1], AF_T.Exp
            )
            ecols = scr.tile([P, BPC], BF16, tag="ecols", bufs=3)
            nc.vector.tensor_scalar_mul(
                ecols[:pt, :],
                blob_sb[:pt, IND_O + t * BPC : IND_O + (t + 1) * BPC],
                e_all[:pt, t : t + 1],
            )
            ecols_tiles[t] = ecols

        # tile-0/1 iaf injects run while w streams in
        inject(0)
        inject(1)

        # --- att_hb = h @ W.T + b_h (bias folded in as a K=1 chunk) ---
        atthb_ps = ps_bc.tile([BPC, A], F32, tag="bc")
        for k in range(KCH):
            nc.tensor.matmul(
                atthb_ps[:],
                blob_sb[:, HT_O + k * BPC : HT_O + (k + 1) * BPC],
                w_sb[:, k * A : (k + 1) * A],
                start=(k == 0),
                stop=False,
            )
        nc.tensor.matmul(
            atthb_ps[:],
            blob_sb[0:1, ONES_O : ONES_O + BPC],
            blob_sb[0:1, BH_O : BH_O + A],
            start=False,
            stop=True,
        )
        atthb_sb = consts.tile([BPC, A], BF16)
        nc.scalar.copy(atthb_sb[:], atthb_ps[:])

        bcmm(0)
        bcmm(1)
        chain_a(0)
        chain_a(1)
        chain_b(0)
        bc_inject(2)

        for t in range(NT):
            pt = _ptile(t)
            if t + 2 < NT:
                chain_a(t + 2)
            if t + 1 < NT:
                chain_b(t + 1)
            if t + 3 < NT:
                bc_inject(t + 3)

            af_g, af_j = af_tiles_sb.pop(t)
            ecols = ecols_tiles.pop(t)
            for c in range(DCH):
                nc.tensor.matmul(
                    acc_ps[:, c, :],
                    ecols[:pt, :],
                    af_g[:pt, af_j * D + c * DC : af_j * D + (c + 1) * DC],
                    start=(t == 0),
                    stop=(t == NT - 1),
                )
            # denominator: sums[b] = sum_rows e_cols[:, b]
            nc.tensor.matmul(
                sums_ps[:],
                ecols[:pt, :],
                ones2_sb[:pt, :],
                start=(t == 0),
                stop=(t == NT - 1),
            )
            # keep the PE clock ramped through the DMA-paced idle window:
            # weight loads are busy-work with no architectural effect (every
            # real matmul self-loads its weights)
            for _f in range(4):
                nc.tensor.ldweights(blob_sb[:, ID_O : ID_O + P])

        # --- normalize and store ---
        recip = consts.tile([BPC, 1], F32)
        nc.vector.reciprocal(recip[:], sums_ps[:, 0:1])
        out_sb = consts.tile([BPC, D], F32)
        for c in range(DCH):
            if c % 2 == 0:
                nc.scalar.mul(out_sb[:, c * DC : (c + 1) * DC], acc_ps[:, c, :], recip[:])
            else:
                nc.vector.tensor_scalar_mul(
                    out_sb[:, c * DC : (c + 1) * DC], acc_ps[:, c, :], recip[:]
                )
            nc.sync.dma_start(
                out[:, c * DC : (c + 1) * DC], out_sb[:, c * DC : (c + 1) * DC]
            )

    nc.compile()
    return nc


_PROGRAM = None


def _get_program():
    global _PROGRAM
    if _PROGRAM is None:
        _PROGRAM = _build_program()
    return _PROGRAM


def _pack_rows(x16, ncols):
    """[R, C] -> [P, NT*C] with dev[p, t*C:(t+1)*C] = x[t*P+p], zero pad."""
    dev = np.zeros((P, NT, ncols), x16.dtype)
    dev[:, :NFULL_T, :] = x16[: NFULL_T * P].reshape(NFULL_T, P, ncols).transpose(1, 0, 2)
    dev[:TAILR, NFULL_T, :] = x16[NFULL_T * P :]
    return np.ascontiguousarray(dev.reshape(P, NT * ncols))


def _host_prep(h, att_feats, internal_att_feats, h2att_w, h2att_b, alpha_w, alpha_b):
    h16 = np.asarray(h).astype(NPBF16)
    af16 = np.asarray(att_feats).astype(NPBF16).reshape(B, L * D)
    iaf16 = np.asarray(internal_att_feats).astype(NPBF16).reshape(B, L * A)
    w16 = np.asarray(h2att_w).astype(NPBF16)                    # [A, D]

    # w_dev[p, k*A+a] = W[a, k*P+p]
    w_dev = np.ascontiguousarray(
        w16.T.reshape(KCH, P, A).transpose(1, 0, 2).reshape(P, KCH * A)
    )

    ind_rows = np.zeros((NT * P, BPC), np.float32)
    rows = np.arange(R)
    ind_rows[rows, rows // L] = 1.0
    ind_t = np.ascontiguousarray(ind_rows[:R].T).astype(NPBF16)  # [BPC, R]
    ind_dev = ind_rows.reshape(NT, P, BPC).transpose(1, 0, 2).reshape(P, NT * BPC)

    blob_common = np.zeros((P, BLOB_C), NPBF16)
    blob_common[:, ABC_O : ABC_O + A] = np.asarray(alpha_w, np.float32).reshape(1, A)
    blob_common[:, ID_O : ID_O + P] = np.eye(P, dtype=NPBF16)
    blob_common[:, IND_O : IND_O + NT * BPC] = ind_dev
    blob_common[0, BH_O : BH_O + A] = np.asarray(h2att_b, np.float32).reshape(A)
    blob_common[0, ONES_O : ONES_O + BPC] = 1.0

    in_maps = []
    for i in range(NCORES):
        sl = slice(i * BPC, (i + 1) * BPC)
        blob_i = blob_common.copy()
        # h_t_dev[p, k*BPC+b] = h[b, k*P+p]
        blob_i[:, HT_O : HT_O + KCH * BPC] = (
            h16[sl].T.reshape(KCH, P, BPC).transpose(1, 0, 2).reshape(P, KCH * BPC)
        )
        in_maps.append(
            {
                "w_t": w_dev,
                "blob": blob_i,
                "ind_t": ind_t,
                "iaf": _pack_rows(iaf16[sl].reshape(R, A), A),
                "af": _pack_rows(af16[sl].reshape(R, D), D),
            }
        )
    return in_maps


def run(trace=False, **inputs):
    """Run the SPMD kernel; returns (full_output [B, D], BassKernelResults)."""
    nc = _get_program()
    in_maps = _host_prep(**inputs)
    res = run_bass_kernel_spmd(nc, in_maps, list(range(NCORES)), trace=trace)
    out = np.concatenate([res.results[i]["out"] for i in range(NCORES)], axis=0)
    return out, res


def kernel(**inputs):
    out, _ = run(trace=False, **inputs)
    return out
